# revision 29
# baseline (speedup 1.0000x reference)
"""Trainium2 Bass kernel for gated GQA attention (nn_Attention_6476810683032).

Sharding: 8 cores = 2 (batch DP) x 4 (head-group TP).
Core c handles batch b=c//4, head group g=c%4 (q-heads 4g..4g+3, kv-head g).
Each core computes a partial o_proj output [D, T] (its 4 heads' contribution,
transposed layout); the host sums the 4 partials per batch and transposes.

On-device per core (all matmuls bf16 with fp32 PSUM accumulation):
  - projections from host-pre-transposed hidden_t [D, T] (channel-major
    outputs for q/gate/k, token-major for v) -- no on-device transposes
  - RMS norm via ones-matmul partition reduction; the rsqrt row is
    partition-broadcast on the Pool engine (not a K=1 PE matmul) and the
    weight multiply is fused into one scalar_tensor_tensor DVE op
  - RoPE via partition-offset elementwise ops with a pre-signed sin table
  - causal attention in transposed-score form: S_T[tk,tq] = k_rot.T@q_rot,
    exp without max subtraction (logits bounded by the RMS norms),
    denominator via ones-matmul; its reciprocal is computed on the [1,CT]
    row and partition-broadcast on Pool (saves the K=1 PE matmul + the
    [128,CT] DVE reciprocal)
  - sigmoid gating fused with the softmax normalization (2 DVE ops)
  - partial o_proj: out_T[dout,t] = wo_slice.T @ gated (bf16 partials,
    summed in f32 on the host)

Scheduling notes (all engines execute their streams in order, so emission
order is the schedule):
  - DMA arrival is ordered to match consumption: small tables on the Pool
    SW queue; wk first on SP; hid tiles alternate SP/ACT in d order with
    the per-d weights (wv, head-0 q/gate blocks) on the opposite queue, so
    phase 0's d-outer loop paces with arrival; remaining q/gate heads and
    wo stream in behind
  - wqq/wqg are stored head-major in DRAM ([NHL*D, HD]) so head-0's
    16 d-tiles (needed in phase 0) are a contiguous early 1 MB
  - per chunk: projection pairs with norm/rope chains sandwiched between
    them, then attention with the two head-pairs' m-loops interleaved
  - o_proj of chunk c-1 (and for chunk 0, the next chunk's first
    projection pair) is drip-fed between attention m-steps as PE filler,
    finishing by ~80% of the m-loop so the PSUM drain clears before the
    next chunk's projections need the banks
  - the final chunk's o_proj is split into head pairs: the (h0,h1) half
    runs as fillers inside the second head-pair's m-loop (partials staged
    in SBUF bf16), so only the (h2,h3) half + add remains after the last
    attention step
  - sigmoids are explicitly ordered after the chunk's norm chains on ACT
    (a scheduler dependency) to avoid ACT function-table reload thrash
"""

import os
import sys
from contextlib import ExitStack

import numpy as np

sys.path.insert(0, "/opt/trn_rl_repo")

import ml_dtypes  # noqa: E402

import concourse.bass as bass  # noqa: E402
import concourse.mybir as mybir  # noqa: E402
import concourse.tile as tile  # noqa: E402
from concourse import bacc  # noqa: E402
from concourse import masks as masks_mod  # noqa: E402

F32 = mybir.dt.float32
BF16 = mybir.dt.bfloat16
AF = mybir.ActivationFunctionType
ALU = mybir.AluOpType
BF = ml_dtypes.bfloat16

P = 128
B, T, D = 2, 2048, 2048
NH, NKV, HD = 16, 4, 128
NHL = NH // NKV          # local q heads per core (4)
CH = 4                   # tq chunks
CT = T // CH             # 512 tokens per chunk
DT = D // P              # 16 contraction tiles
KT = T // P              # 16 tk tiles
EPS = 1e-6
SCALE = HD ** -0.5
N_CORES = 8


def _norm_rope(nc, pools, psr, ones_col, eps_t, x_bf, w_ap,
               cos_sl, sin_sl, out_ap, n):
    """RMS-norm (over partitions) + RoPE on a [128, n] channel-major tile.

    x_bf: [128, n] bf16 SBUF (pre-norm channels-on-partitions tile)
    w_ap: [128, 1] f32 norm weight
    cos_sl/sin_sl: [128, n] bf16 (sin pre-signed: rows 0-63 negated)
    out_ap: [128, n] bf16 destination

    """
    sbw, sbr = pools
    xsq = sbw.tile([P, n], BF16, tag="tmpa", name="xsq")
    nc.vector.tensor_tensor(xsq[:], x_bf, x_bf, op=ALU.mult)
    ssq = psr.tile([1, n], F32, tag="pp", name="ssq")
    nc.tensor.matmul(ssq[:], ones_col, xsq[:], start=True, stop=True)
    rsq = sbr.tile([1, n], BF16, tag="rsq", name="rsq")
    absr = nc.scalar.activation(rsq[:], ssq[:], AF.Abs_reciprocal_sqrt,
                                scale=1.0 / HD, bias=eps_t)
    rbb = sbw.tile([P, n], BF16, tag="rbb", name="rbb", bufs=1)
    nc.gpsimd.partition_broadcast(rbb[:], rsq[:], channels=P)
    xn = sbw.tile([P, n], BF16, tag="xn", name="xn", bufs=2)
    nc.vector.scalar_tensor_tensor(xn[:], rbb[:], w_ap, x_bf,
                                   op0=ALU.mult, op1=ALU.mult)
    t1 = sbw.tile([P, n], BF16, tag="tmpb", name="t1", bufs=2)
    nc.vector.tensor_tensor(t1[:], xn[:], cos_sl, op=ALU.mult)
    h = HD // 2
    xs = sbw.tile([P, n], BF16, tag="tmpc", name="xs")
    nc.vector.tensor_copy(xs[0:h, :], xn[h:P, :])
    nc.vector.tensor_copy(xs[h:P, :], xn[0:h, :])
    t2 = sbw.tile([P, n], BF16, tag="tmpa", name="t2")
    nc.vector.tensor_tensor(t2[:], xs[:], sin_sl, op=ALU.mult)
    nc.vector.tensor_tensor(out_ap, t1[:], t2[:], op=ALU.add)
    return absr


def build_nc():
    nc = bacc.Bacc("TRN2", target_bir_lowering=False, debug=False,
                   num_devices=N_CORES)
    # All weights are host-prepacked into [128, n*128] partition-major
    # layouts so each loads with one (or two) big DMAs.
    hid_d = nc.dram_tensor("hid", [D, T], BF16, kind="ExternalInput")
    wqq_d = nc.dram_tensor("wqq", [P, NHL * DT * HD], BF16,
                           kind="ExternalInput")
    wqg_d = nc.dram_tensor("wqg", [P, NHL * DT * HD], BF16,
                           kind="ExternalInput")
    wk_d = nc.dram_tensor("wk", [P, DT * HD], BF16, kind="ExternalInput")
    wv_d = nc.dram_tensor("wv", [P, DT * HD], BF16, kind="ExternalInput")
    wo_d = nc.dram_tensor("wo", [P, NHL * D], BF16, kind="ExternalInput")
    cos_d = nc.dram_tensor("cost", [P, T], BF16, kind="ExternalInput")
    sin_d = nc.dram_tensor("sinpm", [P, T], BF16, kind="ExternalInput")
    qw_d = nc.dram_tensor("qw", [P, 1], F32, kind="ExternalInput")
    kw_d = nc.dram_tensor("kw", [P, 1], F32, kind="ExternalInput")
    mask_d = nc.dram_tensor("masks", [P, P], BF16, kind="ExternalInput")
    out_d = nc.dram_tensor("out_t", [D, T], BF16, kind="ExternalOutput")
    # final chunk's o_proj head-pair-0 partial; host adds it to out_t
    out0_d = nc.dram_tensor("out0", [D, CT], BF16, kind="ExternalOutput")

    with tile.TileContext(nc) as tc, ExitStack() as ctx, \
            nc.allow_low_precision(reason="bf16 softmax temps validated by rel_err"):
        sbp = ctx.enter_context(tc.tile_pool(name="sbp", bufs=1))
        sbw = ctx.enter_context(tc.tile_pool(name="sbw", bufs=3))
        sbr = ctx.enter_context(tc.tile_pool(name="sbr", bufs=2))
        sbq = ctx.enter_context(tc.tile_pool(name="sbq", bufs=6))
        # PSUM plan (8 banks): psp 2x1 (projections, o_proj fillers, ssq/dn
        # rows), pss 2x2 ([P,2,CT] score pairs / k-proj pairs), psa 2x1
        # (attn accumulators / v-proj).
        psp = ctx.enter_context(tc.tile_pool(name="psp", bufs=2, space="PSUM"))
        pss = ctx.enter_context(tc.tile_pool(name="pss", bufs=2, space="PSUM"))
        psa = ctx.enter_context(tc.tile_pool(name="psa", bufs=2, space="PSUM"))

        # ---- persistent tiles + loads ----
        # Arrival order is engineered: tiny tables via the Pool SW queue;
        # wk first on SP; hid[d] alternates SP/ACT in d order with the per-d
        # small weights on the opposite queue; the rest streams in behind.
        qw = sbp.tile([P, 1], F32, tag="qw")
        nc.gpsimd.dma_start(qw[:], qw_d[:, :])
        kw = sbp.tile([P, 1], F32, tag="kw")
        nc.gpsimd.dma_start(kw[:], kw_d[:, :])
        masks = sbp.tile([P, P], BF16, tag="masks")
        nc.gpsimd.dma_start(masks[:], mask_d[:, :])

        # Packed weight tiles: one [128, n*128] tile per tensor, loaded with
        # few big DMAs (each dma_start costs ~0.7-1.3us of serialized ring
        # time regardless of size; one ring streams ~400 GB/s). Ring plan:
        #   SYNC:   hid evens first (the PE's first need), wo  (then outputs)
        #   SCALAR: hid odds, wqq/wqg heads 1-3  (then output tiles)
        #   POOL:   tiny tables, wk, wv, wqq/wqg head 0, cos, sin
        wk_sb = sbp.tile([P, DT * HD], BF16, tag="wk_sb")
        nc.scalar.dma_start(wk_sb[:], wk_d[:, :])
        wv_sb = sbp.tile([P, DT * HD], BF16, tag="wv_sb")
        wqq_pk = sbp.tile([P, NHL * DT * HD], BF16, tag="wqq_pk")
        wqg_pk = sbp.tile([P, NHL * DT * HD], BF16, tag="wqg_pk")
        nc.gpsimd.dma_start(wqq_pk[:, 0:DT * HD], wqq_d[:, 0:DT * HD])
        nc.gpsimd.dma_start(wqg_pk[:, 0:DT * HD], wqg_d[:, 0:DT * HD])
        cost = sbp.tile([P, T], BF16, tag="cost")
        nc.gpsimd.dma_start(cost[:], cos_d[:, :])
        sinpm = sbp.tile([P, T], BF16, tag="sinpm")
        nc.gpsimd.dma_start(sinpm[:], sin_d[:, :])
        hid = []
        for d in range(DT):
            t = sbp.tile([P, T], BF16, tag=f"hid{d}", name=f"hid{d}")
            hid.append(t)
        nc.sync.dma_start(hid[0][:], hid_d[0:P, :])
        nc.sync.dma_start(wv_sb[:], wv_d[:, :])
        for d in range(2, DT, 2):
            nc.sync.dma_start(hid[d][:], hid_d[d * P:(d + 1) * P, :])
        for d in range(1, DT, 2):
            nc.scalar.dma_start(hid[d][:], hid_d[d * P:(d + 1) * P, :])
        nc.scalar.dma_start(wqq_pk[:, DT * HD:], wqq_d[:, DT * HD:])
        nc.scalar.dma_start(wqg_pk[:, DT * HD:], wqg_d[:, DT * HD:])
        wo_pk = sbp.tile([P, NHL * D], BF16, tag="wo_pk")
        nc.sync.dma_start(wo_pk[:, 0:2 * D], wo_d[:, 0:2 * D])
        nc.sync.dma_start(wo_pk[:, 2 * D:], wo_d[:, 2 * D:])

        def wk(d):
            return wk_sb[:, d * HD:(d + 1) * HD]

        def wv(d):
            return wv_sb[:, d * HD:(d + 1) * HD]

        def wqq_sl(h, d):
            return wqq_pk[:, (h * DT + d) * HD:(h * DT + d + 1) * HD]

        def wqg_sl(h, d):
            return wqg_pk[:, (h * DT + d) * HD:(h * DT + d + 1) * HD]

        def wo_sl(ct4, ds_):
            return wo_pk[:, ct4 * D + ds_.start:ct4 * D + ds_.stop]

        ones_col = sbp.tile([P, 1], BF16, tag="ones_col")
        nc.vector.memset(ones_col[:], 1.0)
        eps_t = sbp.tile([1, 1], F32, tag="eps_t")
        nc.vector.memset(eps_t[:], EPS)
        ident = sbp.tile([P, P], BF16, tag="ident")
        masks_mod.make_identity(nc, ident[:])
        krot = sbp.tile([P, T], BF16, tag="krot")
        vsb = []
        for i in range(KT):
            vsb.append(sbp.tile([P, HD], BF16, tag=f"v{i}", name=f"v{i}"))

        # ---- phase 0: loop A is d-outer over k/v (chunks 0-1) only, pacing
        # the PE with the paired even/odd hid arrival on the sync/scalar
        # rings; the rest (v chunks 2-3, v transposes, chunk-0 head-0 q/gate)
        # follows as loop B, absorbing the DMA tail.
        kps2 = [pss.tile([P, 2, CT], F32, tag="ss", name="kps01"),
                pss.tile([P, 2, CT], F32, tag="ss", name="kps23")]
        vps01 = [psa.tile([P, CT], F32, tag="aa", name="vps0"),
                 psa.tile([P, CT], F32, tag="aa", name="vps1")]
        qp0 = psp.tile([P, CT], F32, tag="pp", name="qp0")
        gp0 = psp.tile([P, CT], F32, tag="pp", name="gp0")
        cs0 = slice(0, CT)
        for d in range(DT):
            st, sp = (d == 0), (d == DT - 1)
            for c in range(CH):
                cs = slice(c * CT, (c + 1) * CT)
                nc.tensor.matmul(kps2[c // 2][:, c % 2, :], wk(d),
                                 hid[d][:, cs], start=st, stop=sp)
            for c in range(2):
                cs = slice(c * CT, (c + 1) * CT)
                nc.tensor.matmul(vps01[c][:], wv(d), hid[d][:, cs],
                                 start=st, stop=sp)
        kbfs = []
        for c in range(CH):
            kbf = sbw.tile([P, CT], BF16, tag="kbf", name="kbf", bufs=4)
            nc.vector.tensor_copy(kbf[:], kps2[c // 2][:, c % 2, :])
            kbfs.append(kbf)
        vct = sbp.tile([P, T], BF16, tag="vct")
        for c in range(2):
            cs = slice(c * CT, (c + 1) * CT)
            nc.vector.tensor_copy(vct[:, cs], vps01[c][:])
        for c in range(2, CH):
            cs = slice(c * CT, (c + 1) * CT)
            ps = psa.tile([P, CT], F32, tag="aa", name="vcps")
            for d in range(DT):
                nc.tensor.matmul(ps[:], wv(d), hid[d][:, cs],
                                 start=(d == 0), stop=(d == DT - 1))
            nc.vector.tensor_copy(vct[:, cs], ps[:])
        for tt in range(KT // 2):
            tps = pss.tile([P, P], BF16, tag="ss", name="tps")
            nc.tensor.transpose(tps[:], vct[:, tt * P:(tt + 1) * P],
                                ident[:])
            nc.vector.tensor_copy(vsb[tt][:], tps[:])
        for d in range(DT):
            nc.tensor.matmul(qp0[:], wqq_sl(0, d), hid[d][:, cs0],
                             start=(d == 0), stop=(d == DT - 1))
        for tt in range(KT // 2, KT):
            tps = pss.tile([P, P], BF16, tag="ss", name="tps")
            nc.tensor.transpose(tps[:], vct[:, tt * P:(tt + 1) * P],
                                ident[:])
            nc.vector.tensor_copy(vsb[tt][:], tps[:])
        for d in range(DT):
            nc.tensor.matmul(gp0[:], wqg_sl(0, d), hid[d][:, cs0],
                             start=(d == 0), stop=(d == DT - 1))
        q_sb0 = sbq.tile([P, CT], BF16, tag="q_sb", bufs=4, name="q_sb0")
        nc.vector.tensor_copy(q_sb0[:], qp0[:])
        g_sb0 = sbq.tile([P, CT], BF16, tag="g_sb", bufs=5, name="g_sb0")
        nc.vector.tensor_copy(g_sb0[:], gp0[:])
        pre_pairs = {0: (q_sb0, g_sb0)}

        # ---- phase 1: per tq-chunk: q/gate proj, attention ----
        # o_proj for chunk c-1 is emitted after chunk c's norm chains so the
        # PE has dense work while the chains' DVE/ACT latency drains.
        def _o_proj_pair1(og):
            """Final chunk: pair-1 accumulation (pair-0 went to out0_d).

            Output tiles drain in dout pairs (one DMA per 2 tiles) and the
            DMAs alternate the sync/scalar rings so the post-attention drain
            is not serialized on a single ring.
            """
            ocs = slice((CH - 1) * CT, CH * CT)
            pools4 = [(psp, "pp"), (pss, "ss"), (psa, "aa")]
            for dt2 in range(DT // 2):
                osb2 = sbw.tile([P, 2, CT], BF16, tag="osb2", bufs=2,
                                name="osb2")
                for j in range(2):
                    dt = 2 * dt2 + j
                    ds_ = slice(dt * P, (dt + 1) * P)
                    pl, tg = pools4[dt % 3]
                    pso = pl.tile([P, CT], F32, tag=tg, name="pso")
                    nc.tensor.matmul(pso[:], wo_sl(2, ds_), og[2][:],
                                     start=True, stop=False)
                    nc.tensor.matmul(pso[:], wo_sl(3, ds_), og[3][:],
                                     start=False, stop=True)
                    if j == 0:
                        nc.vector.tensor_copy(osb2[:, j, :], pso[:])
                    else:
                        nc.scalar.copy(osb2[:, j, :], pso[:])
                dst = out_d[2 * dt2 * P:(2 * dt2 + 2) * P, ocs].rearrange(
                    "(a p) c -> p a c", a=2)
                eng = (nc.sync, nc.scalar, nc.gpsimd)[dt2 % 3]
                eng.dma_start(dst, osb2[:])

        last_exp_ins = None
        prev_gated = None
        for c in range(CH):
            cs = slice(c * CT, (c + 1) * CT)
            q_sbs = {}
            g_sbs = {}
            sigs = []
            qrots = {}

            chain_absr = []

            def _proj(kind, h, cs=None, q_sbs=None, g_sbs=None):
                w_sl = wqq_sl if kind == "q" else wqg_sl
                ps = psp.tile([P, CT], F32, tag="pp")
                for d in range(DT):
                    nc.tensor.matmul(ps[:], w_sl(h, d), hid[d][:, cs],
                                     start=(d == 0), stop=(d == DT - 1))
                if kind == "q":
                    sb = sbq.tile([P, CT], BF16, tag="q_sb", bufs=4)
                else:
                    sb = sbq.tile([P, CT], BF16, tag="g_sb", bufs=5)
                nc.vector.tensor_copy(sb[:], ps[:])
                (q_sbs if kind == "q" else g_sbs)[h] = sb

            def _chain(which, c=None, cs=None, q_sbs=None, qrots=None):
                if which == "k":
                    chain_absr.append(_norm_rope(
                        nc, (sbw, sbr), psp, ones_col[:], eps_t[:],
                        kbfs[c][:], kw[:], cost[:, cs], sinpm[:, cs],
                        krot[:, cs], CT))
                    return
                qrot = sbw.tile([P, CT], BF16, tag="qrot", bufs=4,
                                name="qrot")
                a = _norm_rope(nc, (sbw, sbr), psp, ones_col[:], eps_t[:],
                               q_sbs[which][:], qw[:], cost[:, cs],
                               sinpm[:, cs], qrot[:], CT)
                qrots[which] = qrot
                chain_absr.append(a)

            # q-projections first (gates after), with the chains (k first,
            # then q-chains) interleaved one projection behind: every chain
            # then starts early enough that its ~4.5us cross-engine latency
            # hides behind remaining projections, and the PE stream always
            # LEADS with a dense 16-MM projection (never with a chain's
            # DVE-dependent ssq matmul, which would head-block the in-order
            # PE stream while the previous chunk's gating drains).
            if c in pre_pairs:
                q_sbs[0] = pre_pairs[c][0]
                g_sbs[0] = pre_pairs[c][1]
            projs = [("q", h) for h in range(NHL)
                     if not (h == 0 and c in pre_pairs)]
            projs += [("g", h) for h in range(NHL)
                      if not (h == 0 and c in pre_pairs)]
            chains = ["k", 0, 1, 2, 3]
            _proj(*projs[0], cs=cs, q_sbs=q_sbs, g_sbs=g_sbs)
            for i, ch_ in enumerate(chains):
                _chain(ch_, c=c, cs=cs, q_sbs=q_sbs, qrots=qrots)
                if i + 1 < len(projs):
                    _proj(*projs[i + 1], cs=cs, q_sbs=q_sbs, g_sbs=g_sbs)
            for pt in projs[len(chains) + 1:]:
                _proj(*pt, cs=cs, q_sbs=q_sbs, g_sbs=g_sbs)
            # group this chunk's rsqrt chain after the previous chunk's exps
            # on ACT: the scheduler otherwise hoists the (early-ready) chain
            # into the exp stream, thrashing the ACT function table
            if last_exp_ins is not None:
                bass._add_dep_helper(chain_absr[0].ins, last_exp_ins.ins,
                                     sync=False,
                                     reason="absrsqrt after prev-chunk exps")
            last_sig = None
            for h in range(NHL):
                sig = sbq.tile([P, CT], BF16, tag="sig", bufs=4, name="sig")
                si = nc.scalar.activation(sig[:], g_sbs[h][:], AF.Sigmoid)
                # order sigmoids after the chunk's norm chains on ACT (each
                # function switch reloads the ACT table, ~1.3us)
                bass._add_dep_helper(si.ins, chain_absr[-1].ins, sync=False,
                                     reason="group sigmoids after absrsqrt")
                sigs.append(sig)
                last_sig = si
            gated = []
            nm = 4 * c + 4
            # Filler work drip-fed between attention m-steps keeps the PE
            # dense while ACT runs the exps: o_proj(c-1) tiles; for chunk 0
            # the next chunk's first projection pair; for the final chunk
            # its own o_proj pair-0 halves (during the hp=2 loop only).
            fillers = []
            if prev_gated is not None:
                ocs = slice((c - 1) * CT, c * CT)

                def _mk_oproj(dt, ocs=ocs, og=prev_gated):
                    def run():
                        ds_ = slice(dt * P, (dt + 1) * P)
                        pso = psp.tile([P, CT], F32, tag="pp", name="pso")
                        for ct4 in range(NHL):
                            nc.tensor.matmul(pso[:], wo_sl(ct4, ds_),
                                             og[ct4][:], start=(ct4 == 0),
                                             stop=(ct4 == NHL - 1))
                        osb = sbw.tile([P, CT], BF16, tag="osb", bufs=2,
                                       name="osb")
                        if dt % 2 == 0:
                            nc.vector.tensor_copy(osb[:], pso[:])
                        else:
                            nc.scalar.copy(osb[:], pso[:])
                        nc.sync.dma_start(out_d[ds_, ocs], osb[:])
                    return run
                fillers += [_mk_oproj(dt) for dt in range(DT)]
            if c == 0:
                cs1 = slice(CT, 2 * CT)
                qp1 = psp.tile([P, CT], F32, tag="pp", name="qp1")
                gp1 = psp.tile([P, CT], F32, tag="pp", name="gp1")

                def _mk_proj(ps_t, w_sl, dlist):
                    def run():
                        for d in dlist:
                            nc.tensor.matmul(
                                ps_t[:], w_sl(0, d), hid[d][:, cs1],
                                start=(d == 0), stop=(d == DT - 1))
                    return run
                for d0 in range(0, DT, 4):
                    fillers.append(_mk_proj(qp1, wqq_sl,
                                            range(d0, d0 + 4)))
                for d0 in range(0, DT, 4):
                    fillers.append(_mk_proj(gp1, wqg_sl,
                                            range(d0, d0 + 4)))

                def _pre_cast():
                    q_sb1 = sbq.tile([P, CT], BF16, tag="q_sb", bufs=4,
                                     name="q_sb1")
                    nc.vector.tensor_copy(q_sb1[:], qp1[:])
                    g_sb1 = sbq.tile([P, CT], BF16, tag="g_sb", bufs=5,
                                     name="g_sb1")
                    nc.scalar.copy(g_sb1[:], gp1[:])
                    pre_pairs[1] = (q_sb1, g_sb1)
                fillers.append(_pre_cast)

            # final-chunk pair-0 o_proj fillers (only valid inside hp=2)
            def _mk_pair0(dt):
                def run():
                    ds_ = slice(dt * P, (dt + 1) * P)
                    pso = psp.tile([P, CT], F32, tag="pp", name="pso0")
                    nc.tensor.matmul(pso[:], wo_sl(0, ds_), gated[0][:],
                                     start=True, stop=False)
                    nc.tensor.matmul(pso[:], wo_sl(1, ds_), gated[1][:],
                                     start=False, stop=True)
                    osb = sbw.tile([P, CT], BF16, tag="osb", bufs=2,
                                   name="osb0")
                    if dt % 2 == 0:
                        nc.vector.tensor_copy(osb[:], pso[:])
                    else:
                        nc.scalar.copy(osb[:], pso[:])
                    eng = nc.sync if dt % 2 == 0 else nc.gpsimd
                    eng.dma_start(out0_d[ds_, :], osb[:])
                return run

            chunk_exps = []
            fill = {"i": 0}
            n_steps = 2 * nm

            def _fill_tick(step):
                # finish fillers by ~80% of the m-steps so the last PSUM
                # drain clears before the next chunk's projections
                due = min(len(fillers),
                          len(fillers) * (step + 1) * 5 // (4 * n_steps) + 1)
                while fill["i"] < due:
                    fillers[fill["i"]]()
                    fill["i"] += 1

            step_no = [0]
            for hp in (0, 2):
                if c == CH - 1 and hp == 2:
                    fillers.extend(_mk_pair0(dt) for dt in range(DT))
                pair = (hp, hp + 1)
                # softmax denominators via a bf16 running E accumulator on
                # DVE (one add per m-step covering both heads) + one
                # ones-matmul per head at pair end -- keeps the second E
                # pass off the PE, which is the m-loop's critical engine
                acc2 = sbq.tile([P, 2, CT], BF16, tag="acc2", bufs=1,
                                name="acc2")
                attns = {h: psa.tile([P, CT], F32, tag="aa",
                                     name=f"attn{h}") for h in pair}
                dns = {}
                for m in range(nm):
                    ks = slice(m * P, (m + 1) * P)
                    r = m - 4 * c
                    lo = P * r if r > 0 else 0
                    ns = slice(lo, CT)
                    E2 = sbw.tile([P, 2, CT], BF16, tag="E", name="E2",
                                  bufs=2)
                    sps2 = pss.tile([P, 2, CT], F32, tag="ss", name="sps2")
                    for j, h in enumerate(pair):
                        nc.tensor.matmul(sps2[:, j, ns], krot[:, ks],
                                         qrots[h][:, ns],
                                         start=True, stop=True)
                    if m == nm - 1:
                        # denominator head-start: the bulk ones-matmul over
                        # the accumulator (steps 0..nm-2) overlaps the last
                        # exp; the last step's E is added below
                        for j, h in enumerate(pair):
                            if c == 0:
                                dn = pss.tile([1, CT], F32, tag="ss",
                                              name="dn")
                            else:
                                dn = psp.tile([1, CT], F32, tag="pp",
                                              name="dn")
                            nc.tensor.matmul(dn[:], ones_col[:],
                                             acc2[:, j, :],
                                             start=True, stop=False)
                            dns[h] = dn
                    # one merged exp for both heads (amortizes the ACT
                    # per-op overhead; sps2 spans two adjacent banks)
                    ei = nc.scalar.activation(E2[:, :, ns], sps2[:, :, ns],
                                              AF.Exp, scale=SCALE)
                    chunk_exps.append(ei)
                    if r >= 0:
                        for j in range(2):
                            nc.vector.tensor_tensor(
                                E2[:, j, lo:lo + P], E2[:, j, lo:lo + P],
                                masks[:, 0:P], op=ALU.mult)
                    if m == 0:
                        nc.vector.tensor_copy(acc2[:, :, :], E2[:, :, :])
                    elif m < nm - 1:
                        nc.vector.tensor_tensor(acc2[:, :, ns],
                                                acc2[:, :, ns],
                                                E2[:, :, ns], op=ALU.add)
                    else:
                        for j, h in enumerate(pair):
                            nc.tensor.matmul(dns[h][:, ns], ones_col[:],
                                             E2[:, j, ns],
                                             start=False, stop=True)
                    for j, h in enumerate(pair):
                        nc.tensor.matmul(attns[h][:, ns], vsb[m][:],
                                         E2[:, j, ns], start=(m == 0),
                                         stop=(m == nm - 1))
                    _fill_tick(step_no[0])
                    step_no[0] += 1
                for j, h in enumerate(pair):
                    rcp = sbr.tile([1, CT], F32, tag="rcp", name="rcp")
                    rsc = sbr.tile([1, CT], F32, tag="rsc", name="rsc")
                    nc.vector.reciprocal_approx_accurate(
                        out=rcp[:], in_=dns[h][:], scratch=rsc[:])
                    rcpb = sbr.tile([1, CT], BF16, tag="rcpb", name="rcpb")
                    nc.vector.tensor_copy(rcpb[:], rcp[:])
                    rcb = sbw.tile([P, CT], BF16, tag="rcb", name="rcb",
                                   bufs=2)
                    nc.gpsimd.partition_broadcast(rcb[:], rcpb[:],
                                                  channels=P)
                    tmp = sbw.tile([P, CT], BF16, tag="tmpc", name="tmp")
                    nc.vector.tensor_tensor(tmp[:], attns[h][:], sigs[h][:],
                                            op=ALU.mult)
                    g = sbq.tile([P, CT], BF16, tag="gated", bufs=9)
                    nc.vector.tensor_tensor(g[:], tmp[:], rcb[:],
                                            op=ALU.mult)
                    gated.append(g)

            while fill["i"] < len(fillers):
                fillers[fill["i"]]()
                fill["i"] += 1
            # keep exps after this chunk's sigmoids on ACT (one table load
            # per function block: rsqrt x5, sigmoid x4, exp xN per chunk)
            bass._add_dep_helper(chunk_exps[0].ins, last_sig.ins, sync=False,
                                 reason="exps after sigmoids")
            last_exp_ins = chunk_exps[-1]
            prev_gated = gated
        _o_proj_pair1(prev_gated)
    nc.compile()
    return nc


def _pack_pm(w):
    """[n*128, m] -> [128, n*m]: stack the 128-row tiles along columns."""
    n = w.shape[0] // P
    return np.ascontiguousarray(
        w.reshape(n, P, -1).transpose(1, 0, 2).reshape(P, -1))


def make_in_maps(hidden, cos, sin, wq, wk, wv, wo, q_norm_w, k_norm_w):
    """Build the 8 per-core input maps (host-side sharding + layout prep)."""
    i_idx = np.arange(P)[:, None]
    j_idx = np.arange(P)[None, :]
    masks = (j_idx >= i_idx).astype(BF)
    in_maps = []
    for core in range(N_CORES):
        b, g = core // NKV, core % NKV
        heads = range(NHL * g, NHL * g + NHL)
        sin_t = sin[b].T.copy()
        sin_t[:HD // 2] = -sin_t[:HD // 2]
        in_maps.append({
            "hid": np.ascontiguousarray(hidden[b].T).astype(BF),
            # packed [128, NHL*DT*HD]: head-major then d-tile-major columns
            "wqq": np.concatenate(
                [_pack_pm(wq[:, h * 2 * HD: h * 2 * HD + HD]) for h in heads],
                1).astype(BF),
            "wqg": np.concatenate(
                [_pack_pm(wq[:, h * 2 * HD + HD: (h + 1) * 2 * HD])
                 for h in heads], 1).astype(BF),
            "wk": _pack_pm(wk[:, g * HD:(g + 1) * HD]).astype(BF),
            "wv": _pack_pm(wv[:, g * HD:(g + 1) * HD]).astype(BF),
            "wo": _pack_pm(wo[NHL * HD * g: NHL * HD * (g + 1), :]).astype(BF),
            "cost": np.ascontiguousarray(cos[b].T).astype(BF),
            "sinpm": np.ascontiguousarray(sin_t).astype(BF),
            "qw": np.ascontiguousarray(q_norm_w[:, None]).astype(np.float32),
            "kw": np.ascontiguousarray(k_norm_w[:, None]).astype(np.float32),
            "masks": np.ascontiguousarray(masks),
        })
    return in_maps


def _install_ntff_hook():
    """Inject antenv.axon_hooks with a ctypes NTFF profile hook.

    The container's antenv package lacks axon_hooks, so bass_utils'
    trace=True path can't find the hook. Replicates the boot script's
    _ntff_profile_via_ctypes against libaxon_pjrt.so.
    """
    import contextlib
    import ctypes
    import types

    if "antenv.axon_hooks" in sys.modules:
        return
    lib = None
    for so_path in ("/opt/axon/libaxon_pjrt.so",
                    "/root/.axon_site/axon/libaxon_pjrt.so"):
        try:
            lib = ctypes.CDLL(so_path)
            break
        except OSError:
            continue
    if lib is None:
        return
    if not hasattr(lib, "axon_start_nrt_profile"):
        return
    lib.axon_start_nrt_profile.argtypes = [ctypes.POINTER(ctypes.c_int64),
                                           ctypes.c_size_t]
    lib.axon_start_nrt_profile.restype = ctypes.c_int64
    lib.axon_stop_nrt_profile.argtypes = [ctypes.c_char_p]
    lib.axon_stop_nrt_profile.restype = ctypes.c_int64

    @contextlib.contextmanager
    def _hook(output_dir, device_ids):
        import jax

        jax.devices()
        if device_ids:
            ids = (ctypes.c_int64 * len(device_ids))(*device_ids)
            rc = lib.axon_start_nrt_profile(ids, len(device_ids))
        else:
            rc = lib.axon_start_nrt_profile(None, 0)
        if rc != 0:
            raise RuntimeError(f"axon_start_nrt_profile rc={rc}")
        try:
            yield
        finally:
            n = lib.axon_stop_nrt_profile(str(output_dir).encode())
            print(f"profile: {n} file(s) written to {output_dir}",
                  file=sys.stderr)

    m = types.ModuleType("antenv.axon_hooks")
    m.get_axon_ntff_profile_hook = lambda: _hook
    m.set_axon_ntff_profile_hook = lambda h: None
    sys.modules["antenv.axon_hooks"] = m


_NC_CACHE = None


def _get_nc():
    global _NC_CACHE
    if _NC_CACHE is None:
        _NC_CACHE = build_nc()
    return _NC_CACHE


def kernel(hidden_BTD, cos_BTK, sin_BTK, wq, wk, wv, wo, q_norm_w, k_norm_w,
           segment_ids_BT=None, position_ids_BT=None, **_unused):
    from concourse.bass_utils import run_bass_kernel_spmd

    in_maps = make_in_maps(
        np.asarray(hidden_BTD, np.float32), np.asarray(cos_BTK, np.float32),
        np.asarray(sin_BTK, np.float32), np.asarray(wq, np.float32),
        np.asarray(wk, np.float32), np.asarray(wv, np.float32),
        np.asarray(wo, np.float32), np.asarray(q_norm_w, np.float32),
        np.asarray(k_norm_w, np.float32))
    nc = _get_nc()
    trace = bool(int(os.environ.get("BASS_KERNEL_TRACE", "0")))
    if trace:
        _install_ntff_hook()
    res = run_bass_kernel_spmd(nc, in_maps, core_ids=list(range(N_CORES)),
                               trace=trace)
    out = np.zeros((B, T, D), np.float32)
    for core in range(N_CORES):
        out[core // NKV] += res.results[core]["out_t"].astype(np.float32).T
        out[core // NKV][(CH - 1) * CT:] += \
            res.results[core]["out0"].astype(np.float32).T
    kernel.last_exec_time_ns = res.exec_time_ns
    kernel.last_results = res
    return out


kernel.last_exec_time_ns = None
kernel.last_results = None



# revision 31
# speedup vs baseline: 1.0026x; 1.0026x over previous
"""Trainium2 Bass kernel for gated GQA attention (nn_Attention_6476810683032).

Sharding: 8 cores = 2 (batch DP) x 4 (head-group TP).
Core c handles batch b=c//4, head group g=c%4 (q-heads 4g..4g+3, kv-head g).
Each core computes a partial o_proj output [D, T] (its 4 heads' contribution,
transposed layout); the host sums the 4 partials per batch and transposes.

On-device per core (all matmuls bf16 with fp32 PSUM accumulation):
  - projections from host-pre-transposed hidden_t [D, T] (channel-major
    outputs for q/gate/k, token-major for v) -- no on-device transposes
  - RMS norm via ones-matmul partition reduction; the rsqrt row is
    partition-broadcast on the Pool engine (not a K=1 PE matmul) and the
    weight multiply is fused into one scalar_tensor_tensor DVE op
  - RoPE via partition-offset elementwise ops with a pre-signed sin table
  - causal attention in transposed-score form: S_T[tk,tq] = k_rot.T@q_rot,
    exp without max subtraction (logits bounded by the RMS norms),
    denominator via ones-matmul; its reciprocal is computed on the [1,CT]
    row and partition-broadcast on Pool (saves the K=1 PE matmul + the
    [128,CT] DVE reciprocal)
  - sigmoid gating fused with the softmax normalization (2 DVE ops)
  - partial o_proj: out_T[dout,t] = wo_slice.T @ gated (bf16 partials,
    summed in f32 on the host)

Scheduling notes (all engines execute their streams in order, so emission
order is the schedule):
  - DMA arrival is ordered to match consumption: small tables on the Pool
    SW queue; wk first on SP; hid tiles alternate SP/ACT in d order with
    the per-d weights (wv, head-0 q/gate blocks) on the opposite queue, so
    phase 0's d-outer loop paces with arrival; remaining q/gate heads and
    wo stream in behind
  - wqq/wqg are stored head-major in DRAM ([NHL*D, HD]) so head-0's
    16 d-tiles (needed in phase 0) are a contiguous early 1 MB
  - per chunk: projection pairs with norm/rope chains sandwiched between
    them, then attention with the two head-pairs' m-loops interleaved
  - o_proj of chunk c-1 (and for chunk 0, the next chunk's first
    projection pair) is drip-fed between attention m-steps as PE filler,
    finishing by ~80% of the m-loop so the PSUM drain clears before the
    next chunk's projections need the banks
  - the final chunk's o_proj is split into head pairs: the (h0,h1) half
    runs as fillers inside the second head-pair's m-loop (partials staged
    in SBUF bf16), so only the (h2,h3) half + add remains after the last
    attention step
  - sigmoids are explicitly ordered after the chunk's norm chains on ACT
    (a scheduler dependency) to avoid ACT function-table reload thrash
"""

import os
import sys
from contextlib import ExitStack

import numpy as np

sys.path.insert(0, "/opt/trn_rl_repo")

import ml_dtypes  # noqa: E402

import concourse.bass as bass  # noqa: E402
import concourse.mybir as mybir  # noqa: E402
import concourse.tile as tile  # noqa: E402
from concourse import bacc  # noqa: E402
from concourse import masks as masks_mod  # noqa: E402

F32 = mybir.dt.float32
BF16 = mybir.dt.bfloat16
AF = mybir.ActivationFunctionType
ALU = mybir.AluOpType
BF = ml_dtypes.bfloat16

P = 128
B, T, D = 2, 2048, 2048
NH, NKV, HD = 16, 4, 128
NHL = NH // NKV          # local q heads per core (4)
CH = 4                   # tq chunks
CT = T // CH             # 512 tokens per chunk
DT = D // P              # 16 contraction tiles
KT = T // P              # 16 tk tiles
EPS = 1e-6
SCALE = HD ** -0.5
N_CORES = 8


def _norm_rope(nc, pools, psr, ones_col, eps_t, x_bf, w_ap,
               cos_sl, sin_sl, out_ap, n):
    """RMS-norm (over partitions) + RoPE on a [128, n] channel-major tile.

    x_bf: [128, n] bf16 SBUF (pre-norm channels-on-partitions tile)
    w_ap: [128, 1] f32 norm weight
    cos_sl/sin_sl: [128, n] bf16 (sin pre-signed: rows 0-63 negated)
    out_ap: [128, n] bf16 destination

    """
    sbw, sbr = pools
    xsq = sbw.tile([P, n], BF16, tag="tmpa", name="xsq")
    nc.vector.tensor_tensor(xsq[:], x_bf, x_bf, op=ALU.mult)
    ssq = psr.tile([1, n], F32, tag="pp", name="ssq")
    nc.tensor.matmul(ssq[:], ones_col, xsq[:], start=True, stop=True)
    rsq = sbr.tile([1, n], BF16, tag="rsq", name="rsq")
    absr = nc.scalar.activation(rsq[:], ssq[:], AF.Abs_reciprocal_sqrt,
                                scale=1.0 / HD, bias=eps_t)
    rbb = sbw.tile([P, n], BF16, tag="rbb", name="rbb", bufs=1)
    nc.gpsimd.partition_broadcast(rbb[:], rsq[:], channels=P)
    xn = sbw.tile([P, n], BF16, tag="xn", name="xn", bufs=2)
    nc.vector.scalar_tensor_tensor(xn[:], rbb[:], w_ap, x_bf,
                                   op0=ALU.mult, op1=ALU.mult)
    t1 = sbw.tile([P, n], BF16, tag="tmpb", name="t1", bufs=2)
    nc.vector.tensor_tensor(t1[:], xn[:], cos_sl, op=ALU.mult)
    h = HD // 2
    xs = sbw.tile([P, n], BF16, tag="tmpc", name="xs")
    nc.vector.tensor_copy(xs[0:h, :], xn[h:P, :])
    nc.vector.tensor_copy(xs[h:P, :], xn[0:h, :])
    t2 = sbw.tile([P, n], BF16, tag="tmpa", name="t2")
    nc.vector.tensor_tensor(t2[:], xs[:], sin_sl, op=ALU.mult)
    nc.vector.tensor_tensor(out_ap, t1[:], t2[:], op=ALU.add)
    return absr


def build_nc():
    nc = bacc.Bacc("TRN2", target_bir_lowering=False, debug=False,
                   num_devices=N_CORES)
    # All weights are host-prepacked into [128, n*128] partition-major
    # layouts so each loads with one (or two) big DMAs.
    hid_d = nc.dram_tensor("hid", [D, T], BF16, kind="ExternalInput")
    wqq_d = nc.dram_tensor("wqq", [P, NHL * DT * HD], BF16,
                           kind="ExternalInput")
    wqg_d = nc.dram_tensor("wqg", [P, NHL * DT * HD], BF16,
                           kind="ExternalInput")
    wk_d = nc.dram_tensor("wk", [P, DT * HD], BF16, kind="ExternalInput")
    wv_d = nc.dram_tensor("wv", [P, DT * HD], BF16, kind="ExternalInput")
    wo_d = nc.dram_tensor("wo", [P, NHL * D], BF16, kind="ExternalInput")
    cos_d = nc.dram_tensor("cost", [P, T], BF16, kind="ExternalInput")
    sin_d = nc.dram_tensor("sinpm", [P, T], BF16, kind="ExternalInput")
    qw_d = nc.dram_tensor("qw", [P, 1], F32, kind="ExternalInput")
    kw_d = nc.dram_tensor("kw", [P, 1], F32, kind="ExternalInput")
    mask_d = nc.dram_tensor("masks", [P, P], BF16, kind="ExternalInput")
    out_d = nc.dram_tensor("out_t", [D, T], BF16, kind="ExternalOutput")
    # final chunk's o_proj head-pair-0 partial; host adds it to out_t
    out0_d = nc.dram_tensor("out0", [D, CT], BF16, kind="ExternalOutput")

    with tile.TileContext(nc) as tc, ExitStack() as ctx, \
            nc.allow_low_precision(reason="bf16 softmax temps validated by rel_err"):
        sbp = ctx.enter_context(tc.tile_pool(name="sbp", bufs=1))
        sbw = ctx.enter_context(tc.tile_pool(name="sbw", bufs=3))
        sbr = ctx.enter_context(tc.tile_pool(name="sbr", bufs=2))
        sbq = ctx.enter_context(tc.tile_pool(name="sbq", bufs=6))
        # PSUM plan (8 banks): psp 2x1 (projections, o_proj fillers, ssq/dn
        # rows), pss 2x2 ([P,2,CT] score pairs / k-proj pairs), psa 2x1
        # (attn accumulators / v-proj).
        psp = ctx.enter_context(tc.tile_pool(name="psp", bufs=2, space="PSUM"))
        pss = ctx.enter_context(tc.tile_pool(name="pss", bufs=2, space="PSUM"))
        psa = ctx.enter_context(tc.tile_pool(name="psa", bufs=2, space="PSUM"))

        # ---- persistent tiles + loads ----
        # Arrival order is engineered: tiny tables via the Pool SW queue;
        # wk first on SP; hid[d] alternates SP/ACT in d order with the per-d
        # small weights on the opposite queue; the rest streams in behind.
        qw = sbp.tile([P, 1], F32, tag="qw")
        nc.gpsimd.dma_start(qw[:], qw_d[:, :])
        kw = sbp.tile([P, 1], F32, tag="kw")
        nc.gpsimd.dma_start(kw[:], kw_d[:, :])
        masks = sbp.tile([P, P], BF16, tag="masks")
        nc.gpsimd.dma_start(masks[:], mask_d[:, :])

        # Packed weight tiles: one [128, n*128] tile per tensor, loaded with
        # few big DMAs (each dma_start costs ~0.7-1.3us of serialized ring
        # time regardless of size; one ring streams ~400 GB/s). Ring plan:
        #   SYNC:   hid evens first (the PE's first need), wo  (then outputs)
        #   SCALAR: hid odds, wqq/wqg heads 1-3  (then output tiles)
        #   POOL:   tiny tables, wk, wv, wqq/wqg head 0, cos, sin
        wk_sb = sbp.tile([P, DT * HD], BF16, tag="wk_sb")
        nc.scalar.dma_start(wk_sb[:], wk_d[:, :])
        wv_sb = sbp.tile([P, DT * HD], BF16, tag="wv_sb")
        wqq_pk = sbp.tile([P, NHL * DT * HD], BF16, tag="wqq_pk")
        wqg_pk = sbp.tile([P, NHL * DT * HD], BF16, tag="wqg_pk")
        nc.gpsimd.dma_start(wqq_pk[:, 0:DT * HD], wqq_d[:, 0:DT * HD])
        nc.gpsimd.dma_start(wqg_pk[:, 0:DT * HD], wqg_d[:, 0:DT * HD])
        cost = sbp.tile([P, T], BF16, tag="cost")
        nc.gpsimd.dma_start(cost[:], cos_d[:, :])
        sinpm = sbp.tile([P, T], BF16, tag="sinpm")
        nc.gpsimd.dma_start(sinpm[:], sin_d[:, :])
        hid = []
        for d in range(DT):
            t = sbp.tile([P, T], BF16, tag=f"hid{d}", name=f"hid{d}")
            hid.append(t)
        nc.sync.dma_start(hid[0][:], hid_d[0:P, :])
        nc.sync.dma_start(wv_sb[:], wv_d[:, :])
        for d in range(2, DT, 2):
            nc.sync.dma_start(hid[d][:], hid_d[d * P:(d + 1) * P, :])
        for d in range(1, DT, 2):
            nc.scalar.dma_start(hid[d][:], hid_d[d * P:(d + 1) * P, :])
        nc.scalar.dma_start(wqq_pk[:, DT * HD:], wqq_d[:, DT * HD:])
        nc.scalar.dma_start(wqg_pk[:, DT * HD:], wqg_d[:, DT * HD:])
        wo_pk = sbp.tile([P, NHL * D], BF16, tag="wo_pk")
        nc.sync.dma_start(wo_pk[:, 0:2 * D], wo_d[:, 0:2 * D])
        nc.sync.dma_start(wo_pk[:, 2 * D:], wo_d[:, 2 * D:])

        def wk(d):
            return wk_sb[:, d * HD:(d + 1) * HD]

        def wv(d):
            return wv_sb[:, d * HD:(d + 1) * HD]

        def wqq_sl(h, d):
            return wqq_pk[:, (h * DT + d) * HD:(h * DT + d + 1) * HD]

        def wqg_sl(h, d):
            return wqg_pk[:, (h * DT + d) * HD:(h * DT + d + 1) * HD]

        def wo_sl(ct4, ds_):
            return wo_pk[:, ct4 * D + ds_.start:ct4 * D + ds_.stop]

        ones_col = sbp.tile([P, 1], BF16, tag="ones_col")
        nc.vector.memset(ones_col[:], 1.0)
        eps_t = sbp.tile([1, 1], F32, tag="eps_t")
        nc.vector.memset(eps_t[:], EPS)
        ident = sbp.tile([P, P], BF16, tag="ident")
        masks_mod.make_identity(nc, ident[:])
        krot = sbp.tile([P, T], BF16, tag="krot")
        vsb = []
        for i in range(KT):
            vsb.append(sbp.tile([P, HD], BF16, tag=f"v{i}", name=f"v{i}"))

        # ---- phase 0: loop A is d-outer over k/v (chunks 0-1) only, pacing
        # the PE with the paired even/odd hid arrival on the sync/scalar
        # rings; the rest (v chunks 2-3, v transposes, chunk-0 head-0 q/gate)
        # follows as loop B, absorbing the DMA tail.
        kps2 = [pss.tile([P, 2, CT], F32, tag="ss", name="kps01"),
                pss.tile([P, 2, CT], F32, tag="ss", name="kps23")]
        vps01 = [psa.tile([P, CT], F32, tag="aa", name="vps0"),
                 psa.tile([P, CT], F32, tag="aa", name="vps1")]
        qp0 = psp.tile([P, CT], F32, tag="pp", name="qp0")
        gp0 = psp.tile([P, CT], F32, tag="pp", name="gp0")
        cs0 = slice(0, CT)
        for d in range(DT):
            st, sp = (d == 0), (d == DT - 1)
            for c in range(CH):
                cs = slice(c * CT, (c + 1) * CT)
                nc.tensor.matmul(kps2[c // 2][:, c % 2, :], wk(d),
                                 hid[d][:, cs], start=st, stop=sp)
            for c in range(2):
                cs = slice(c * CT, (c + 1) * CT)
                nc.tensor.matmul(vps01[c][:], wv(d), hid[d][:, cs],
                                 start=st, stop=sp)
        kbfs = []
        for c in range(CH):
            kbf = sbw.tile([P, CT], BF16, tag="kbf", name="kbf", bufs=4)
            nc.vector.tensor_copy(kbf[:], kps2[c // 2][:, c % 2, :])
            kbfs.append(kbf)
        vct = sbp.tile([P, T], BF16, tag="vct")
        for c in range(2):
            cs = slice(c * CT, (c + 1) * CT)
            nc.vector.tensor_copy(vct[:, cs], vps01[c][:])
        for c in range(2, CH):
            cs = slice(c * CT, (c + 1) * CT)
            ps = psa.tile([P, CT], F32, tag="aa", name="vcps")
            for d in range(DT):
                nc.tensor.matmul(ps[:], wv(d), hid[d][:, cs],
                                 start=(d == 0), stop=(d == DT - 1))
            nc.vector.tensor_copy(vct[:, cs], ps[:])
        for tt in range(KT // 2):
            tps = pss.tile([P, P], BF16, tag="ss", name="tps")
            nc.tensor.transpose(tps[:], vct[:, tt * P:(tt + 1) * P],
                                ident[:])
            nc.vector.tensor_copy(vsb[tt][:], tps[:])
        for d in range(DT):
            nc.tensor.matmul(qp0[:], wqq_sl(0, d), hid[d][:, cs0],
                             start=(d == 0), stop=(d == DT - 1))
        # all four chunks' k norm/rope chains run here: kbf and the rope
        # tables are already resident, and hoisting them off the chunk
        # boundaries removes their ssq matmuls from the PE stream right
        # when the previous chunk's gating is still draining on DVE
        for c in range(CH):
            cs = slice(c * CT, (c + 1) * CT)
            _norm_rope(nc, (sbw, sbr), psp, ones_col[:], eps_t[:],
                       kbfs[c][:], kw[:], cost[:, cs], sinpm[:, cs],
                       krot[:, cs], CT)
        for tt in range(KT // 2, KT):
            tps = pss.tile([P, P], BF16, tag="ss", name="tps")
            nc.tensor.transpose(tps[:], vct[:, tt * P:(tt + 1) * P],
                                ident[:])
            nc.vector.tensor_copy(vsb[tt][:], tps[:])
        for d in range(DT):
            nc.tensor.matmul(gp0[:], wqg_sl(0, d), hid[d][:, cs0],
                             start=(d == 0), stop=(d == DT - 1))
        q_sb0 = sbq.tile([P, CT], BF16, tag="q_sb", bufs=4, name="q_sb0")
        nc.vector.tensor_copy(q_sb0[:], qp0[:])
        g_sb0 = sbq.tile([P, CT], BF16, tag="g_sb", bufs=5, name="g_sb0")
        nc.vector.tensor_copy(g_sb0[:], gp0[:])
        pre_pairs = {0: (q_sb0, g_sb0)}

        # ---- phase 1: per tq-chunk: q/gate proj, attention ----
        # o_proj for chunk c-1 is emitted after chunk c's norm chains so the
        # PE has dense work while the chains' DVE/ACT latency drains.
        def _o_proj_pair1(og):
            """Final chunk: pair-1 accumulation (pair-0 went to out0_d).

            Output tiles drain in dout pairs (one DMA per 2 tiles) and the
            DMAs alternate the sync/scalar rings so the post-attention drain
            is not serialized on a single ring.
            """
            ocs = slice((CH - 1) * CT, CH * CT)
            pools4 = [(psp, "pp"), (pss, "ss"), (psa, "aa")]
            for dt2 in range(DT // 2):
                osb2 = sbw.tile([P, 2, CT], BF16, tag="osb2", bufs=2,
                                name="osb2")
                for j in range(2):
                    dt = 2 * dt2 + j
                    ds_ = slice(dt * P, (dt + 1) * P)
                    pl, tg = pools4[dt % 3]
                    pso = pl.tile([P, CT], F32, tag=tg, name="pso")
                    nc.tensor.matmul(pso[:], wo_sl(2, ds_), og[2][:],
                                     start=True, stop=False)
                    nc.tensor.matmul(pso[:], wo_sl(3, ds_), og[3][:],
                                     start=False, stop=True)
                    if j == 0:
                        nc.vector.tensor_copy(osb2[:, j, :], pso[:])
                    else:
                        nc.scalar.copy(osb2[:, j, :], pso[:])
                dst = out_d[2 * dt2 * P:(2 * dt2 + 2) * P, ocs].rearrange(
                    "(a p) c -> p a c", a=2)
                eng = (nc.sync, nc.scalar, nc.gpsimd)[dt2 % 3]
                eng.dma_start(dst, osb2[:])

        last_exp_ins = None
        prev_gated = None
        for c in range(CH):
            cs = slice(c * CT, (c + 1) * CT)
            q_sbs = {}
            g_sbs = {}
            sigs = []
            qrots = {}

            chain_absr = []

            def _proj(kind, h, cs=None, q_sbs=None, g_sbs=None):
                w_sl = wqq_sl if kind == "q" else wqg_sl
                ps = psp.tile([P, CT], F32, tag="pp")
                for d in range(DT):
                    nc.tensor.matmul(ps[:], w_sl(h, d), hid[d][:, cs],
                                     start=(d == 0), stop=(d == DT - 1))
                if kind == "q":
                    sb = sbq.tile([P, CT], BF16, tag="q_sb", bufs=4)
                else:
                    sb = sbq.tile([P, CT], BF16, tag="g_sb", bufs=5)
                nc.vector.tensor_copy(sb[:], ps[:])
                (q_sbs if kind == "q" else g_sbs)[h] = sb

            def _chain(which, c=None, cs=None, q_sbs=None, qrots=None):
                qrot = sbw.tile([P, CT], BF16, tag="qrot", bufs=4,
                                name="qrot")
                a = _norm_rope(nc, (sbw, sbr), psp, ones_col[:], eps_t[:],
                               q_sbs[which][:], qw[:], cost[:, cs],
                               sinpm[:, cs], qrot[:], CT)
                qrots[which] = qrot
                chain_absr.append(a)

            # q-projections first (gates after), with the chains (k first,
            # then q-chains) interleaved one projection behind: every chain
            # then starts early enough that its ~4.5us cross-engine latency
            # hides behind remaining projections, and the PE stream always
            # LEADS with a dense 16-MM projection (never with a chain's
            # DVE-dependent ssq matmul, which would head-block the in-order
            # PE stream while the previous chunk's gating drains).
            if c in pre_pairs:
                q_sbs[0] = pre_pairs[c][0]
                g_sbs[0] = pre_pairs[c][1]
            projs = [("q", h) for h in range(NHL)
                     if not (h == 0 and c in pre_pairs)]
            projs += [("g", h) for h in range(NHL)
                      if not (h == 0 and c in pre_pairs)]
            chains = [0, 1, 2, 3]
            _proj(*projs[0], cs=cs, q_sbs=q_sbs, g_sbs=g_sbs)
            _proj(*projs[1], cs=cs, q_sbs=q_sbs, g_sbs=g_sbs)
            for i, ch_ in enumerate(chains):
                _chain(ch_, c=c, cs=cs, q_sbs=q_sbs, qrots=qrots)
                if i + 2 < len(projs):
                    _proj(*projs[i + 2], cs=cs, q_sbs=q_sbs, g_sbs=g_sbs)
            for pt in projs[len(chains) + 2:]:
                _proj(*pt, cs=cs, q_sbs=q_sbs, g_sbs=g_sbs)
            # group this chunk's rsqrt chain after the previous chunk's exps
            # on ACT: the scheduler otherwise hoists the (early-ready) chain
            # into the exp stream, thrashing the ACT function table
            if last_exp_ins is not None:
                bass._add_dep_helper(chain_absr[0].ins, last_exp_ins.ins,
                                     sync=False,
                                     reason="absrsqrt after prev-chunk exps")
            last_sig = None
            for h in range(NHL):
                sig = sbq.tile([P, CT], BF16, tag="sig", bufs=4, name="sig")
                si = nc.scalar.activation(sig[:], g_sbs[h][:], AF.Sigmoid)
                # order sigmoids after the chunk's norm chains on ACT (each
                # function switch reloads the ACT table, ~1.3us)
                bass._add_dep_helper(si.ins, chain_absr[-1].ins, sync=False,
                                     reason="group sigmoids after absrsqrt")
                sigs.append(sig)
                last_sig = si
            gated = []
            nm = 4 * c + 4
            # Filler work drip-fed between attention m-steps keeps the PE
            # dense while ACT runs the exps: o_proj(c-1) tiles; for chunk 0
            # the next chunk's first projection pair; for the final chunk
            # its own o_proj pair-0 halves (during the hp=2 loop only).
            fillers = []
            if prev_gated is not None:
                ocs = slice((c - 1) * CT, c * CT)

                def _mk_oproj(dt, ocs=ocs, og=prev_gated):
                    def run():
                        ds_ = slice(dt * P, (dt + 1) * P)
                        pso = psp.tile([P, CT], F32, tag="pp", name="pso")
                        for ct4 in range(NHL):
                            nc.tensor.matmul(pso[:], wo_sl(ct4, ds_),
                                             og[ct4][:], start=(ct4 == 0),
                                             stop=(ct4 == NHL - 1))
                        osb = sbw.tile([P, CT], BF16, tag="osb", bufs=2,
                                       name="osb")
                        if dt % 2 == 0:
                            nc.vector.tensor_copy(osb[:], pso[:])
                        else:
                            nc.scalar.copy(osb[:], pso[:])
                        nc.sync.dma_start(out_d[ds_, ocs], osb[:])
                    return run
                fillers += [_mk_oproj(dt) for dt in range(DT)]
            if c == 0:
                cs1 = slice(CT, 2 * CT)
                qp1 = psp.tile([P, CT], F32, tag="pp", name="qp1")
                gp1 = psp.tile([P, CT], F32, tag="pp", name="gp1")

                def _mk_proj(ps_t, w_sl, dlist):
                    def run():
                        for d in dlist:
                            nc.tensor.matmul(
                                ps_t[:], w_sl(0, d), hid[d][:, cs1],
                                start=(d == 0), stop=(d == DT - 1))
                    return run
                for d0 in range(0, DT, 4):
                    fillers.append(_mk_proj(qp1, wqq_sl,
                                            range(d0, d0 + 4)))
                for d0 in range(0, DT, 4):
                    fillers.append(_mk_proj(gp1, wqg_sl,
                                            range(d0, d0 + 4)))

                def _pre_cast():
                    q_sb1 = sbq.tile([P, CT], BF16, tag="q_sb", bufs=4,
                                     name="q_sb1")
                    nc.vector.tensor_copy(q_sb1[:], qp1[:])
                    g_sb1 = sbq.tile([P, CT], BF16, tag="g_sb", bufs=5,
                                     name="g_sb1")
                    nc.scalar.copy(g_sb1[:], gp1[:])
                    pre_pairs[1] = (q_sb1, g_sb1)
                fillers.append(_pre_cast)

            # final-chunk pair-0 o_proj fillers (only valid inside hp=2)
            def _mk_pair0(dt):
                def run():
                    ds_ = slice(dt * P, (dt + 1) * P)
                    pso = psp.tile([P, CT], F32, tag="pp", name="pso0")
                    nc.tensor.matmul(pso[:], wo_sl(0, ds_), gated[0][:],
                                     start=True, stop=False)
                    nc.tensor.matmul(pso[:], wo_sl(1, ds_), gated[1][:],
                                     start=False, stop=True)
                    osb = sbw.tile([P, CT], BF16, tag="osb", bufs=2,
                                   name="osb0")
                    if dt % 2 == 0:
                        nc.vector.tensor_copy(osb[:], pso[:])
                    else:
                        nc.scalar.copy(osb[:], pso[:])
                    eng = nc.sync if dt % 2 == 0 else nc.gpsimd
                    eng.dma_start(out0_d[ds_, :], osb[:])
                return run

            chunk_exps = []
            fill = {"i": 0}
            n_steps = 2 * nm

            def _fill_tick(step):
                # finish fillers by ~80% of the m-steps so the last PSUM
                # drain clears before the next chunk's projections
                due = min(len(fillers),
                          len(fillers) * (step + 1) * 5 // (4 * n_steps) + 1)
                while fill["i"] < due:
                    fillers[fill["i"]]()
                    fill["i"] += 1

            step_no = [0]
            for hp in (0, 2):
                if c == CH - 1 and hp == 2:
                    fillers.extend(_mk_pair0(dt) for dt in range(DT))
                pair = (hp, hp + 1)
                # softmax denominators via a bf16 running E accumulator on
                # DVE (one add per m-step covering both heads) + one
                # ones-matmul per head at pair end -- keeps the second E
                # pass off the PE, which is the m-loop's critical engine
                acc2 = sbq.tile([P, 2, CT], BF16, tag="acc2", bufs=1,
                                name="acc2")
                attns = {h: psa.tile([P, CT], F32, tag="aa",
                                     name=f"attn{h}") for h in pair}
                dns = {}
                for m in range(nm):
                    ks = slice(m * P, (m + 1) * P)
                    r = m - 4 * c
                    lo = P * r if r > 0 else 0
                    ns = slice(lo, CT)
                    E2 = sbw.tile([P, 2, CT], BF16, tag="E", name="E2",
                                  bufs=2)
                    sps2 = pss.tile([P, 2, CT], F32, tag="ss", name="sps2")
                    for j, h in enumerate(pair):
                        nc.tensor.matmul(sps2[:, j, ns], krot[:, ks],
                                         qrots[h][:, ns],
                                         start=True, stop=True)
                    if m == nm - 1:
                        # denominator head-start: the bulk ones-matmul over
                        # the accumulator (steps 0..nm-2) overlaps the last
                        # exp; the last step's E is added below
                        for j, h in enumerate(pair):
                            dn = pss.tile([1, CT], F32, tag="ss", name="dn")
                            nc.tensor.matmul(dn[:], ones_col[:],
                                             acc2[:, j, :],
                                             start=True, stop=False)
                            dns[h] = dn
                    # one merged exp for both heads (amortizes the ACT
                    # per-op overhead; sps2 spans two adjacent banks)
                    ei = nc.scalar.activation(E2[:, :, ns], sps2[:, :, ns],
                                              AF.Exp, scale=SCALE)
                    chunk_exps.append(ei)
                    if r >= 0:
                        for j in range(2):
                            nc.vector.tensor_tensor(
                                E2[:, j, lo:lo + P], E2[:, j, lo:lo + P],
                                masks[:, 0:P], op=ALU.mult)
                    if m == 0:
                        nc.vector.tensor_copy(acc2[:, :, :], E2[:, :, :])
                    elif m < nm - 1:
                        nc.vector.tensor_tensor(acc2[:, :, ns],
                                                acc2[:, :, ns],
                                                E2[:, :, ns], op=ALU.add)
                    else:
                        for j, h in enumerate(pair):
                            nc.tensor.matmul(dns[h][:, ns], ones_col[:],
                                             E2[:, j, ns],
                                             start=False, stop=True)
                    for j, h in enumerate(pair):
                        nc.tensor.matmul(attns[h][:, ns], vsb[m][:],
                                         E2[:, j, ns], start=(m == 0),
                                         stop=(m == nm - 1))
                    _fill_tick(step_no[0])
                    step_no[0] += 1
                for j, h in enumerate(pair):
                    rcp = sbr.tile([1, CT], F32, tag="rcp", name="rcp")
                    rsc = sbr.tile([1, CT], F32, tag="rsc", name="rsc")
                    nc.vector.reciprocal_approx_accurate(
                        out=rcp[:], in_=dns[h][:], scratch=rsc[:])
                    rcpb = sbr.tile([1, CT], BF16, tag="rcpb", name="rcpb")
                    nc.vector.tensor_copy(rcpb[:], rcp[:])
                    rcb = sbw.tile([P, CT], BF16, tag="rcb", name="rcb",
                                   bufs=2)
                    nc.gpsimd.partition_broadcast(rcb[:], rcpb[:],
                                                  channels=P)
                    tmp = sbw.tile([P, CT], BF16, tag="tmpc", name="tmp")
                    nc.vector.tensor_tensor(tmp[:], attns[h][:], sigs[h][:],
                                            op=ALU.mult)
                    g = sbq.tile([P, CT], BF16, tag="gated", bufs=9)
                    nc.vector.tensor_tensor(g[:], tmp[:], rcb[:],
                                            op=ALU.mult)
                    gated.append(g)

            while fill["i"] < len(fillers):
                fillers[fill["i"]]()
                fill["i"] += 1
            # keep exps after this chunk's sigmoids on ACT (one table load
            # per function block: rsqrt x5, sigmoid x4, exp xN per chunk)
            bass._add_dep_helper(chunk_exps[0].ins, last_sig.ins, sync=False,
                                 reason="exps after sigmoids")
            last_exp_ins = chunk_exps[-1]
            prev_gated = gated
        _o_proj_pair1(prev_gated)
    nc.compile()
    return nc


def _pack_pm(w):
    """[n*128, m] -> [128, n*m]: stack the 128-row tiles along columns."""
    n = w.shape[0] // P
    return np.ascontiguousarray(
        w.reshape(n, P, -1).transpose(1, 0, 2).reshape(P, -1))


def make_in_maps(hidden, cos, sin, wq, wk, wv, wo, q_norm_w, k_norm_w):
    """Build the 8 per-core input maps (host-side sharding + layout prep)."""
    i_idx = np.arange(P)[:, None]
    j_idx = np.arange(P)[None, :]
    masks = (j_idx >= i_idx).astype(BF)
    in_maps = []
    for core in range(N_CORES):
        b, g = core // NKV, core % NKV
        heads = range(NHL * g, NHL * g + NHL)
        sin_t = sin[b].T.copy()
        sin_t[:HD // 2] = -sin_t[:HD // 2]
        in_maps.append({
            "hid": np.ascontiguousarray(hidden[b].T).astype(BF),
            # packed [128, NHL*DT*HD]: head-major then d-tile-major columns
            "wqq": np.concatenate(
                [_pack_pm(wq[:, h * 2 * HD: h * 2 * HD + HD]) for h in heads],
                1).astype(BF),
            "wqg": np.concatenate(
                [_pack_pm(wq[:, h * 2 * HD + HD: (h + 1) * 2 * HD])
                 for h in heads], 1).astype(BF),
            "wk": _pack_pm(wk[:, g * HD:(g + 1) * HD]).astype(BF),
            "wv": _pack_pm(wv[:, g * HD:(g + 1) * HD]).astype(BF),
            "wo": _pack_pm(wo[NHL * HD * g: NHL * HD * (g + 1), :]).astype(BF),
            "cost": np.ascontiguousarray(cos[b].T).astype(BF),
            "sinpm": np.ascontiguousarray(sin_t).astype(BF),
            "qw": np.ascontiguousarray(q_norm_w[:, None]).astype(np.float32),
            "kw": np.ascontiguousarray(k_norm_w[:, None]).astype(np.float32),
            "masks": np.ascontiguousarray(masks),
        })
    return in_maps


def _install_ntff_hook():
    """Inject antenv.axon_hooks with a ctypes NTFF profile hook.

    The container's antenv package lacks axon_hooks, so bass_utils'
    trace=True path can't find the hook. Replicates the boot script's
    _ntff_profile_via_ctypes against libaxon_pjrt.so.
    """
    import contextlib
    import ctypes
    import types

    if "antenv.axon_hooks" in sys.modules:
        return
    lib = None
    for so_path in ("/opt/axon/libaxon_pjrt.so",
                    "/root/.axon_site/axon/libaxon_pjrt.so"):
        try:
            lib = ctypes.CDLL(so_path)
            break
        except OSError:
            continue
    if lib is None:
        return
    if not hasattr(lib, "axon_start_nrt_profile"):
        return
    lib.axon_start_nrt_profile.argtypes = [ctypes.POINTER(ctypes.c_int64),
                                           ctypes.c_size_t]
    lib.axon_start_nrt_profile.restype = ctypes.c_int64
    lib.axon_stop_nrt_profile.argtypes = [ctypes.c_char_p]
    lib.axon_stop_nrt_profile.restype = ctypes.c_int64

    @contextlib.contextmanager
    def _hook(output_dir, device_ids):
        import jax

        jax.devices()
        if device_ids:
            ids = (ctypes.c_int64 * len(device_ids))(*device_ids)
            rc = lib.axon_start_nrt_profile(ids, len(device_ids))
        else:
            rc = lib.axon_start_nrt_profile(None, 0)
        if rc != 0:
            raise RuntimeError(f"axon_start_nrt_profile rc={rc}")
        try:
            yield
        finally:
            n = lib.axon_stop_nrt_profile(str(output_dir).encode())
            print(f"profile: {n} file(s) written to {output_dir}",
                  file=sys.stderr)

    m = types.ModuleType("antenv.axon_hooks")
    m.get_axon_ntff_profile_hook = lambda: _hook
    m.set_axon_ntff_profile_hook = lambda h: None
    sys.modules["antenv.axon_hooks"] = m


_NC_CACHE = None


def _get_nc():
    global _NC_CACHE
    if _NC_CACHE is None:
        _NC_CACHE = build_nc()
    return _NC_CACHE


def kernel(hidden_BTD, cos_BTK, sin_BTK, wq, wk, wv, wo, q_norm_w, k_norm_w,
           segment_ids_BT=None, position_ids_BT=None, **_unused):
    from concourse.bass_utils import run_bass_kernel_spmd

    in_maps = make_in_maps(
        np.asarray(hidden_BTD, np.float32), np.asarray(cos_BTK, np.float32),
        np.asarray(sin_BTK, np.float32), np.asarray(wq, np.float32),
        np.asarray(wk, np.float32), np.asarray(wv, np.float32),
        np.asarray(wo, np.float32), np.asarray(q_norm_w, np.float32),
        np.asarray(k_norm_w, np.float32))
    nc = _get_nc()
    trace = bool(int(os.environ.get("BASS_KERNEL_TRACE", "0")))
    if trace:
        _install_ntff_hook()
    res = run_bass_kernel_spmd(nc, in_maps, core_ids=list(range(N_CORES)),
                               trace=trace)
    out = np.zeros((B, T, D), np.float32)
    for core in range(N_CORES):
        out[core // NKV] += res.results[core]["out_t"].astype(np.float32).T
        out[core // NKV][(CH - 1) * CT:] += \
            res.results[core]["out0"].astype(np.float32).T
    kernel.last_exec_time_ns = res.exec_time_ns
    kernel.last_results = res
    return out


kernel.last_exec_time_ns = None
kernel.last_results = None

